# revision 19
# baseline (speedup 1.0000x reference)
"""Trainium2 Bass kernel for nn_BiLinearDotLayer.

Computes, for feature (B,F,E)=(2048,200,64) f32 and weight (F,E,E):
    bilinear[b,i,d] = sum_e feature[b,i,e] * weight[i,e,d]
    out[b,i,j]      = sum_d bilinear[b,i,d] * feature[b,j,d]

Strategy (8 NeuronCores, data-parallel over batch):
  - Each core handles 256 batches; weight replicated.
  - Host pre-transposes feature to featT[e, b, i] and packs even/odd
    batches into SBUF partition halves (p*64+e) so all on-chip tiles use
    128 partitions (full DMA width) and the two batch parities run
    concurrently on the PE array's row-strips (K=64 each).
  - Both einsums run fully on-chip per block of 128 batches; only the
    feature shard and weight are read and only the final (256,200,200)
    f32 output is written per core.
  - Matmuls run in float32r (single-pass fp32, ~1e-4 rel rounding);
    einsum2's moving operand is padded to N=256 for the 1 cycle/row
    fp32r fast path.
"""

import os
import sys

for _p in ("/opt/trn_rl_repo", "/root/.axon_site/_ro/trn_rl_repo"):
    if os.path.isdir(_p) and _p not in sys.path:
        sys.path.insert(0, _p)

import numpy as np

B, F, E = 2048, 200, 64
NCORES = 8
BLOC = B // NCORES            # 256 batches per core
NPAIR = BLOC // 2             # 128 even/odd batch pairs per core
BLOCKS = 4
PPB = NPAIR // BLOCKS         # 32 pairs (64 batches) per block
IGRP = 16                     # einsum1 i's per PSUM group (1 bank)
FPAD = 64                     # ftile column padding so e2 rhs can read N=256
WSPLIT = 4                    # weight resident in 4 tiles (50 i's each)
STG = 4                       # pairs per staged out-DMA (8 batches, 1.28MB)

USE_F32R = True               # single-pass fp32r for einsum2 (e1 stays exact fp32:
                              # fp32r requires output base partition 0, which the
                              # parity-1 einsum1 matmuls can't satisfy)

_RUNNER = None


def _build_program():
    import concourse.tile as tile
    from concourse import bacc, mybir

    f32 = mybir.dt.float32
    mmdt = mybir.dt.float32r if USE_F32R else f32
    nc = bacc.Bacc("TRN2", target_bir_lowering=False, debug=False)

    fpk = nc.dram_tensor("fpk", [128, NPAIR * F], mmdt, kind="ExternalInput")
    wpk = nc.dram_tensor("wpk", [128, F * E], f32, kind="ExternalInput")
    # Device-friendly output layout: out_dev[p, b, ci, j] = out[b, 2p+ci, j].
    # Each partition's slice is contiguous in DRAM, so out-DMA descriptors are
    # one 12.8KB run per partition per stage group (vs 800B interleaved runs).
    # The host un-permutes afterwards.
    out = nc.dram_tensor("out", [100, BLOC, 2, F], f32, kind="ExternalOutput")
    out_v = out.ap()

    FW = F // WSPLIT  # i's per resident weight tile

    with tile.TileContext(nc) as tc:
        with (
            tc.tile_pool(name="wpool", bufs=1) as wpool,
            tc.tile_pool(name="fpool", bufs=2) as fpool,
            tc.tile_pool(name="bpool", bufs=2) as bpool,
            tc.tile_pool(name="stpool", bufs=2) as stpool,
            tc.tile_pool(name="ps1", bufs=3, space="PSUM") as ps1pool,
            tc.tile_pool(name="ps2", bufs=2, space="PSUM") as ps2pool,
        ):
            # weight resident, split so early matmuls don't wait on the full load
            wtiles = []
            for j in range(WSPLIT):
                wt = wpool.tile([128, FW * E], f32, name=f"w{j}", tag=f"w{j}")
                nc.sync.dma_start(out=wt[:], in_=wpk.ap()[:, j * FW * E : (j + 1) * FW * E])
                wtiles.append(wt)

            ftiles = [None] * BLOCKS
            btiles = [None] * BLOCKS

            def load_block(k):
                ftiles[k] = fpool.tile([128, PPB * F + FPAD], mmdt, name="ftile", tag="ftile")
                nc.sync.dma_start(
                    out=ftiles[k][:, : PPB * F],
                    in_=fpk.ap()[:, k * PPB * F : (k + 1) * PPB * F],
                )
                btiles[k] = bpool.tile([128, PPB * F], mmdt, name="btile", tag="btile")

            cpy = 0  # alternates einsum1 copies between DVE and ACT
            dma_i = 0  # alternates out-DMAs between sync and scalar rings

            def e1_group(k, i0):
                """einsum1 for i in [i0, i0+gs) of block k."""
                nonlocal cpy
                gs = min(IGRP, F - i0)
                f3 = ftiles[k][:, : PPB * F].rearrange("p (bb i) -> p bb i", i=F)
                pst = ps1pool.tile([128, IGRP * PPB], f32)
                for g in range(gs):
                    i = i0 + g
                    wt = wtiles[i // FW]
                    io = i % FW
                    for p in (0, 1):
                        nc.tensor.matmul(
                            out=pst[p * 64 : (p + 1) * 64, g * PPB : (g + 1) * PPB],
                            lhsT=wt[p * 64 : (p + 1) * 64, io * E : (io + 1) * E],
                            rhs=f3[p * 64 : (p + 1) * 64, :, i].bitcast(f32),
                            start=True,
                            stop=True,
                        )
                src = pst[:, : gs * PPB].rearrange("p (g bb) -> p g bb", bb=PPB)
                dst = btiles[k][:].rearrange("p (bb i) -> p i bb", i=F)[:, i0 : i0 + gs, :]
                if cpy % 2 == 0:
                    nc.vector.tensor_copy(out=dst, in_=src)
                else:
                    nc.scalar.copy(out=dst, in_=src)
                cpy += 1

            def e2_stage_group(k, m):
                """einsum2 for pairs [m, m+STG) of block k + staged out-DMA.

                fp32r fast path wants N>=256: stream 256 rhs columns, keep 200.
                i-chunks are stride-2 interleaved (ci = i%2) so out partition p
                owns rows i=2p, 2p+1 of each out[b]."""
                nonlocal dma_i
                ftile, btile = ftiles[k], btiles[k]
                bt4 = btile[:].rearrange("p (bb i2 ci) -> p bb ci i2", i2=100, ci=2)
                stage = stpool.tile([128, STG * 4 * F], f32)
                for u in range(STG):
                    bb = m + u
                    psA = ps2pool.tile([128, 512], f32)
                    psB = ps2pool.tile([128, 512], f32)
                    for ci in (0, 1):
                        for p, pst2 in ((0, psA), (1, psB)):
                            nc.tensor.matmul(
                                out=pst2[0:100, ci * 256 : ci * 256 + 256],
                                lhsT=bt4[p * 64 : (p + 1) * 64, bb, ci, :],
                                rhs=ftile[
                                    p * 64 : (p + 1) * 64, bb * F : bb * F + 256
                                ],
                                start=True,
                                stop=True,
                            )
                    # both parity copies run concurrently on different engines
                    for p, pst2, eng in (
                        (0, psA, nc.vector.tensor_copy),
                        (1, psB, nc.scalar.copy),
                    ):
                        src = pst2[0:100].rearrange("q (ci j) -> q ci j", ci=2)[
                            :, :, 0:F
                        ]
                        b_loc = u * 2 + p
                        dst = stage[
                            0:100, b_loc * 2 * F : (b_loc + 1) * 2 * F
                        ].rearrange("q (ci j) -> q ci j", ci=2)
                        eng(out=dst, in_=src)
                b0 = k * 2 * PPB + 2 * m
                dma_eng = nc.sync if dma_i % 2 == 0 else nc.scalar
                dma_i += 1
                dma_eng.dma_start(
                    out=out_v[:, b0 : b0 + 2 * STG, :, :],
                    in_=stage[0:100, :].rearrange(
                        "p (b ci j) -> p b ci j", ci=2, j=F
                    ),
                )

            # Software-pipelined schedule: einsum1 of block k is emitted
            # interleaved with einsum2 of block k-1 so the PE instruction
            # stream stays dense (keeps the HAM clock-gate warm) and e2's
            # dependency latency hides under e1's matmul work.
            load_block(0)
            n_groups = (F + IGRP - 1) // IGRP
            e2_ms = list(range(0, PPB, STG))
            for k in range(BLOCKS + 1):
                if k + 1 < BLOCKS:
                    load_block(k + 1)
                g_i = 0
                m_i = 0
                while (k < BLOCKS and g_i < n_groups) or (k > 0 and m_i < len(e2_ms)):
                    if k < BLOCKS and g_i < n_groups:
                        e1_group(k, g_i * IGRP)
                        g_i += 1
                    if k > 0:
                        # spread e2 stage-groups of the previous block evenly
                        # across this block's e1 groups
                        target = (
                            len(e2_ms)
                            if k == BLOCKS or g_i >= n_groups
                            else (g_i * len(e2_ms)) // n_groups
                        )
                        while m_i < min(target, len(e2_ms)):
                            e2_stage_group(k - 1, e2_ms[m_i])
                            m_i += 1

    nc.compile()
    return nc


class _Runner:
    """Builds the program once and keeps a reusable sharded jit."""

    def __init__(self):
        self.nc = _build_program()
        import jax
        from jax.sharding import Mesh, PartitionSpec
        from jax.experimental.shard_map import shard_map
        from concourse import mybir
        from concourse import bass2jax

        bass2jax.install_neuronx_cc_hook()
        nc = self.nc

        partition_name = (
            nc.partition_id_tensor.name if nc.partition_id_tensor else None
        )
        in_names, out_names, out_avals, zero_outs = [], [], [], []
        for alloc in nc.m.functions[0].allocations:
            if not isinstance(alloc, mybir.MemoryLocationSet):
                continue
            name = alloc.memorylocations[0].name
            if alloc.kind == "ExternalInput":
                if name != partition_name:
                    in_names.append(name)
            elif alloc.kind == "ExternalOutput":
                shape = tuple(alloc.tensor_shape)
                dtype = mybir.dt.np(alloc.dtype)
                out_names.append(name)
                out_avals.append(jax.core.ShapedArray(shape, dtype))
                zero_outs.append(np.zeros(shape, dtype))
        self.in_names = list(in_names)
        self.out_names = out_names
        self.out_avals = out_avals
        self.zero_outs = zero_outs
        n_params = len(in_names)
        n_outs = len(out_avals)
        in_names_full = in_names + out_names
        if partition_name is not None:
            in_names_full.append(partition_name)
        donate = tuple(range(n_params, n_params + n_outs))

        def _body(*args):
            operands = list(args)
            if partition_name is not None:
                operands.append(bass2jax.partition_id_tensor())
            outs = bass2jax._bass_exec_p.bind(
                *operands,
                out_avals=tuple(out_avals),
                in_names=tuple(in_names_full),
                out_names=tuple(out_names),
                lowering_input_output_aliases=(),
                sim_require_finite=True,
                sim_require_nnan=True,
                nc=nc,
            )
            return tuple(outs)

        devices = jax.devices()[:NCORES]
        mesh = Mesh(np.asarray(devices), ("core",))
        in_specs = (PartitionSpec("core"),) * (n_params + n_outs)
        out_specs = (PartitionSpec("core"),) * n_outs
        self.sharded = jax.jit(
            shard_map(
                _body,
                mesh=mesh,
                in_specs=in_specs,
                out_specs=out_specs,
                check_rep=False,
            ),
            donate_argnums=donate,
            keep_unused=True,
        )

    def run(self, concat_inputs):
        """concat_inputs: dict name -> (8*shape0, ...) array."""
        args = [concat_inputs[n] for n in self.in_names]
        zeros = [
            np.zeros((NCORES * z.shape[0], *z.shape[1:]), z.dtype)
            for z in self.zero_outs
        ]
        outs = self.sharded(*args, *zeros)
        return {n: np.asarray(outs[i]) for i, n in enumerate(self.out_names)}


def _get_runner():
    global _RUNNER
    if _RUNNER is None:
        _RUNNER = _Runner()
    return _RUNNER


def pack_inputs(feature, weight):
    """Host-side packing: returns dict of concatenated per-core inputs."""
    feature = np.ascontiguousarray(np.asarray(feature, dtype=np.float32))
    weight = np.ascontiguousarray(np.asarray(weight, dtype=np.float32))
    # featT pack: fpk[core][p*64+e, bb*F+i] = feature[core*BLOC + 2*bb + p, i, e]
    ft = feature.reshape(NCORES, NPAIR, 2, F, E)  # [core, bb, p, i, e]
    fpk = np.ascontiguousarray(ft.transpose(0, 2, 4, 1, 3)).reshape(
        NCORES * 128, NPAIR * F
    )
    wt = np.ascontiguousarray(weight.transpose(1, 0, 2)).reshape(E, F * E)
    wpk_one = np.concatenate([wt, wt], axis=0)  # (128, F*E)
    wpk = np.tile(wpk_one, (NCORES, 1))
    return {"fpk": fpk, "wpk": wpk}


def kernel(feature, weight):
    r = _get_runner()
    ins = pack_inputs(feature, weight)
    outs = r.run(ins)
    return unpack_output(outs["out"])


def unpack_output(out_dev):
    """out_dev: (8*100, BLOC, 2, F) device layout -> (B, F, F)."""
    o = out_dev.reshape(NCORES, 100, BLOC, 2, F)
    # out[core, b, 2p+ci, j] = o[core, p, b, ci, j]
    return np.ascontiguousarray(o.transpose(0, 2, 1, 3, 4)).reshape(B, F, F)


if __name__ == "__main__":
    rng = np.random.default_rng(0)
    feature = rng.standard_normal((B, F, E), dtype=np.float32)
    weight = (0.01 * rng.standard_normal((F, E, E))).astype(np.float32)
    got = kernel(feature, weight)
    bil = np.einsum("bie,ied->bid", feature.astype(np.float64), weight.astype(np.float64))
    ref = np.einsum("bid,bjd->bij", bil, feature.astype(np.float64))
    err = np.abs(got - ref)
    denom = np.abs(ref).max()
    print("max abs err:", err.max(), "rel(scale):", err.max() / denom)
    l2 = np.linalg.norm((got - ref).ravel()) / np.linalg.norm(ref.ravel())
    print("L2 rel:", l2)


# revision 22
# speedup vs baseline: 1.1849x; 1.1849x over previous
"""Trainium2 Bass kernel for nn_BiLinearDotLayer.

Computes, for feature (B,F,E)=(2048,200,64) f32 and weight (F,E,E):
    bilinear[b,i,d] = sum_e feature[b,i,e] * weight[i,e,d]
    out[b,i,j]      = sum_d bilinear[b,i,d] * feature[b,j,d]

Strategy (8 NeuronCores, data-parallel over batch):
  - Each core handles 256 batches; weight replicated.
  - Host pre-transposes feature to featT[e, b, i] and packs even/odd
    batches into SBUF partition halves (p*64+e) so all on-chip tiles use
    128 partitions (full DMA width) and the two batch parities run
    concurrently on the PE array's row-strips (K=64 each).
  - Both einsums run fully on-chip per block of 128 batches; only the
    feature shard and weight are read and only the final (256,200,200)
    f32 output is written per core.
  - Matmuls run in float32r (single-pass fp32, ~1e-4 rel rounding);
    einsum2's moving operand is padded to N=256 for the 1 cycle/row
    fp32r fast path.
"""

import os
import sys

for _p in ("/opt/trn_rl_repo", "/root/.axon_site/_ro/trn_rl_repo"):
    if os.path.isdir(_p) and _p not in sys.path:
        sys.path.insert(0, _p)

import numpy as np

B, F, E = 2048, 200, 64
NCORES = 8
BLOC = B // NCORES            # 256 batches per core
NPAIR = BLOC // 2             # 128 even/odd batch pairs per core
BLOCKS = 2
PPB = NPAIR // BLOCKS         # 64 pairs (128 batches) per block
IGRP = 8                      # einsum1 i's per PSUM group (1 bank)
FPAD = 64                     # ftile column padding so e2 rhs can read N=256
STG = 2                       # pairs per staged out-DMA (4 batches, 640KB)

USE_F32R = True               # single-pass fp32r for einsum2 (e1 stays exact fp32:
                              # fp32r requires output base partition 0, which the
                              # parity-1 einsum1 matmuls can't satisfy)

_RUNNER = None


def _build_program():
    import concourse.tile as tile
    from concourse import bacc, mybir

    f32 = mybir.dt.float32
    mmdt = mybir.dt.float32r if USE_F32R else f32
    nc = bacc.Bacc("TRN2", target_bir_lowering=False, debug=False)

    fpk = nc.dram_tensor("fpk", [128, NPAIR * F], mmdt, kind="ExternalInput")
    wpk = nc.dram_tensor("wpk", [128, F * E], f32, kind="ExternalInput")
    # Device-friendly output layout: out_dev[p, b, ci, j] = out[b, 2p+ci, j].
    # Each partition's slice is contiguous in DRAM, so out-DMA descriptors are
    # one 12.8KB run per partition per stage group (vs 800B interleaved runs).
    # The host un-permutes afterwards.
    out = nc.dram_tensor("out", [100, BLOC, 2, F], f32, kind="ExternalOutput")
    out_v = out.ap()

    with tile.TileContext(nc) as tc:
        with (
            tc.tile_pool(name="wpool", bufs=3) as wpool,
            tc.tile_pool(name="fpool", bufs=2) as fpool,
            tc.tile_pool(name="bpool", bufs=1) as bpool,
            tc.tile_pool(name="stpool", bufs=4) as stpool,
            tc.tile_pool(name="ps1", bufs=2, space="PSUM") as ps1pool,
            tc.tile_pool(name="ps2", bufs=3, space="PSUM") as ps2pool,
        ):
            ftiles = [None] * BLOCKS
            btiles = [None] * BLOCKS

            def load_block(k):
                ftiles[k] = fpool.tile([128, PPB * F + FPAD], mmdt, name="ftile", tag="ftile")
                nc.sync.dma_start(
                    out=ftiles[k][:, : PPB * F],
                    in_=fpk.ap()[:, k * PPB * F : (k + 1) * PPB * F],
                )
                btiles[k] = bpool.tile([128, PPB * F], mmdt, name="btile", tag="btile")

            cpy = 0  # alternates einsum1 copies between DVE and ACT
            dma_i = 0  # alternates out-DMAs between sync and scalar rings

            def e1_group(k, i0):
                """einsum1 for i in [i0, i0+gs) of block k."""
                nonlocal cpy
                gs = min(IGRP, F - i0)
                f3 = ftiles[k][:, : PPB * F].rearrange("p (bb i) -> p bb i", i=F)
                wseg = wpool.tile([128, IGRP * E], f32, name="wseg", tag="wseg")
                nc.sync.dma_start(
                    out=wseg[:, : gs * E], in_=wpk.ap()[:, i0 * E : (i0 + gs) * E]
                )
                pst = ps1pool.tile([128, IGRP * PPB], f32)
                for g in range(gs):
                    i = i0 + g
                    for p in (0, 1):
                        nc.tensor.matmul(
                            out=pst[p * 64 : (p + 1) * 64, g * PPB : (g + 1) * PPB],
                            lhsT=wseg[p * 64 : (p + 1) * 64, g * E : (g + 1) * E],
                            rhs=f3[p * 64 : (p + 1) * 64, :, i].bitcast(f32),
                            start=True,
                            stop=True,
                        )
                src = pst[:, : gs * PPB].rearrange("p (g bb) -> p g bb", bb=PPB)
                dst = btiles[k][:].rearrange("p (bb i) -> p i bb", i=F)[:, i0 : i0 + gs, :]
                if cpy % 2 == 0:
                    nc.vector.tensor_copy(out=dst, in_=src)
                else:
                    nc.scalar.copy(out=dst, in_=src)
                cpy += 1

            def e2_stage_group(k, m):
                """einsum2 for pairs [m, m+STG) of block k + staged out-DMA.

                fp32r fast path wants N>=256: stream 256 rhs columns, keep 200.
                i-chunks are stride-2 interleaved (ci = i%2) so out partition p
                owns rows i=2p, 2p+1 of each out[b]."""
                nonlocal dma_i
                ftile, btile = ftiles[k], btiles[k]
                bt4 = btile[:].rearrange("p (bb i2 ci) -> p bb ci i2", i2=100, ci=2)
                stage = stpool.tile([128, STG * 4 * F], f32)
                for u in range(STG):
                    bb = m + u
                    psA = ps2pool.tile([128, 512], f32)
                    psB = ps2pool.tile([128, 512], f32)
                    for ci in (0, 1):
                        for p, pst2 in ((0, psA), (1, psB)):
                            nc.tensor.matmul(
                                out=pst2[0:100, ci * 256 : ci * 256 + 256],
                                lhsT=bt4[p * 64 : (p + 1) * 64, bb, ci, :],
                                rhs=ftile[
                                    p * 64 : (p + 1) * 64, bb * F : bb * F + 256
                                ],
                                start=True,
                                stop=True,
                            )
                    # both parity copies run concurrently on different engines
                    for p, pst2, eng in (
                        (0, psA, nc.vector.tensor_copy),
                        (1, psB, nc.scalar.copy),
                    ):
                        src = pst2[0:100].rearrange("q (ci j) -> q ci j", ci=2)[
                            :, :, 0:F
                        ]
                        b_loc = u * 2 + p
                        dst = stage[
                            0:100, b_loc * 2 * F : (b_loc + 1) * 2 * F
                        ].rearrange("q (ci j) -> q ci j", ci=2)
                        eng(out=dst, in_=src)
                b0 = k * 2 * PPB + 2 * m
                dma_eng = nc.sync if dma_i % 2 == 0 else nc.scalar
                dma_i += 1
                dma_eng.dma_start(
                    out=out_v[:, b0 : b0 + 2 * STG, :, :],
                    in_=stage[0:100, :].rearrange(
                        "p (b ci j) -> p b ci j", ci=2, j=F
                    ),
                )

            # Software-pipelined schedule: einsum1 of block k is emitted
            # interleaved with einsum2 of block k-1 so the PE instruction
            # stream stays dense (keeps the HAM clock-gate warm) and e2's
            # dependency latency hides under e1's matmul work.
            load_block(0)
            n_groups = (F + IGRP - 1) // IGRP
            e2_ms = list(range(0, PPB, STG))
            for k in range(BLOCKS + 1):
                if k + 1 < BLOCKS:
                    load_block(k + 1)
                g_i = 0
                m_i = 0
                while (k < BLOCKS and g_i < n_groups) or (k > 0 and m_i < len(e2_ms)):
                    if k < BLOCKS and g_i < n_groups:
                        e1_group(k, g_i * IGRP)
                        g_i += 1
                    if k > 0:
                        # spread e2 stage-groups of the previous block evenly
                        # across this block's e1 groups
                        target = (
                            len(e2_ms)
                            if k == BLOCKS or g_i >= n_groups
                            else (g_i * len(e2_ms)) // n_groups
                        )
                        while m_i < min(target, len(e2_ms)):
                            e2_stage_group(k - 1, e2_ms[m_i])
                            m_i += 1

    nc.compile()
    return nc


class _Runner:
    """Builds the program once and keeps a reusable sharded jit."""

    def __init__(self):
        self.nc = _build_program()
        import jax
        from jax.sharding import Mesh, PartitionSpec
        from jax.experimental.shard_map import shard_map
        from concourse import mybir
        from concourse import bass2jax

        bass2jax.install_neuronx_cc_hook()
        nc = self.nc

        partition_name = (
            nc.partition_id_tensor.name if nc.partition_id_tensor else None
        )
        in_names, out_names, out_avals, zero_outs = [], [], [], []
        for alloc in nc.m.functions[0].allocations:
            if not isinstance(alloc, mybir.MemoryLocationSet):
                continue
            name = alloc.memorylocations[0].name
            if alloc.kind == "ExternalInput":
                if name != partition_name:
                    in_names.append(name)
            elif alloc.kind == "ExternalOutput":
                shape = tuple(alloc.tensor_shape)
                dtype = mybir.dt.np(alloc.dtype)
                out_names.append(name)
                out_avals.append(jax.core.ShapedArray(shape, dtype))
                zero_outs.append(np.zeros(shape, dtype))
        self.in_names = list(in_names)
        self.out_names = out_names
        self.out_avals = out_avals
        self.zero_outs = zero_outs
        n_params = len(in_names)
        n_outs = len(out_avals)
        in_names_full = in_names + out_names
        if partition_name is not None:
            in_names_full.append(partition_name)
        donate = tuple(range(n_params, n_params + n_outs))

        def _body(*args):
            operands = list(args)
            if partition_name is not None:
                operands.append(bass2jax.partition_id_tensor())
            outs = bass2jax._bass_exec_p.bind(
                *operands,
                out_avals=tuple(out_avals),
                in_names=tuple(in_names_full),
                out_names=tuple(out_names),
                lowering_input_output_aliases=(),
                sim_require_finite=True,
                sim_require_nnan=True,
                nc=nc,
            )
            return tuple(outs)

        devices = jax.devices()[:NCORES]
        mesh = Mesh(np.asarray(devices), ("core",))
        in_specs = (PartitionSpec("core"),) * (n_params + n_outs)
        out_specs = (PartitionSpec("core"),) * n_outs
        self.sharded = jax.jit(
            shard_map(
                _body,
                mesh=mesh,
                in_specs=in_specs,
                out_specs=out_specs,
                check_rep=False,
            ),
            donate_argnums=donate,
            keep_unused=True,
        )

    def run(self, concat_inputs):
        """concat_inputs: dict name -> (8*shape0, ...) array."""
        args = [concat_inputs[n] for n in self.in_names]
        zeros = [
            np.zeros((NCORES * z.shape[0], *z.shape[1:]), z.dtype)
            for z in self.zero_outs
        ]
        outs = self.sharded(*args, *zeros)
        return {n: np.asarray(outs[i]) for i, n in enumerate(self.out_names)}


def _get_runner():
    global _RUNNER
    if _RUNNER is None:
        _RUNNER = _Runner()
    return _RUNNER


def pack_inputs(feature, weight):
    """Host-side packing: returns dict of concatenated per-core inputs."""
    feature = np.ascontiguousarray(np.asarray(feature, dtype=np.float32))
    weight = np.ascontiguousarray(np.asarray(weight, dtype=np.float32))
    # featT pack: fpk[core][p*64+e, bb*F+i] = feature[core*BLOC + 2*bb + p, i, e]
    ft = feature.reshape(NCORES, NPAIR, 2, F, E)  # [core, bb, p, i, e]
    fpk = np.ascontiguousarray(ft.transpose(0, 2, 4, 1, 3)).reshape(
        NCORES * 128, NPAIR * F
    )
    wt = np.ascontiguousarray(weight.transpose(1, 0, 2)).reshape(E, F * E)
    wpk_one = np.concatenate([wt, wt], axis=0)  # (128, F*E)
    wpk = np.tile(wpk_one, (NCORES, 1))
    return {"fpk": fpk, "wpk": wpk}


def kernel(feature, weight):
    r = _get_runner()
    ins = pack_inputs(feature, weight)
    outs = r.run(ins)
    return unpack_output(outs["out"])


def unpack_output(out_dev):
    """out_dev: (8*100, BLOC, 2, F) device layout -> (B, F, F)."""
    o = out_dev.reshape(NCORES, 100, BLOC, 2, F)
    # out[core, b, 2p+ci, j] = o[core, p, b, ci, j]
    return np.ascontiguousarray(o.transpose(0, 2, 1, 3, 4)).reshape(B, F, F)


if __name__ == "__main__":
    rng = np.random.default_rng(0)
    feature = rng.standard_normal((B, F, E), dtype=np.float32)
    weight = (0.01 * rng.standard_normal((F, E, E))).astype(np.float32)
    got = kernel(feature, weight)
    bil = np.einsum("bie,ied->bid", feature.astype(np.float64), weight.astype(np.float64))
    ref = np.einsum("bid,bjd->bij", bil, feature.astype(np.float64))
    err = np.abs(got - ref)
    denom = np.abs(ref).max()
    print("max abs err:", err.max(), "rel(scale):", err.max() / denom)
    l2 = np.linalg.norm((got - ref).ravel()) / np.linalg.norm(ref.ravel())
    print("L2 rel:", l2)
